# revision 36
# baseline (speedup 1.0000x reference)
"""Trainium2 Bass kernel for nn_MultiHeadAttention_T_4818953306886.

Reference semantics (B=8, S=2048, D=64, H=8, HD=512):
    q = (x @ Wq + bq).reshape(B*H, S, D)      # raw row-major view!
    k, v likewise
    attn = softmax(q @ k^T / sqrt(D), axis=2)
    ctx = attn @ v                             # [B*H, S, D]
    out = ctx.reshape(B, 1, S, HD) @ Wo + bo   # [B, 1, S, D]
    y = LayerNorm(x[:, None] + out) * gamma + beta

The raw reshape means head h's local position i maps to projection row
h*256 + i//8 and column (i%8)*64 + d.  We enumerate head-local positions
c-major as i' = c*256 + r (c = i%8 block, r = i//8); softmax/PV are
invariant to that shared permutation of the key/value index, and the
out-projection row s' = h*256 + r is exactly the original sequence row.

Per-core dataflow (one batch per core, weights replicated), fp8 attention:
  xt   [65, 2048] f16   [x^T ; ones] - the ones row turns the weight
                        matrices' appended bias row into a fused +bias
  wq/wk/wv [65, 512] f16 (k-bias dropped: softmax-invariant)
  Q/K/V projections run in f16 on the PE; the psum evacuations quantize
  to fp8e4: Qt [64 d, h, c, r], Kt [64 d, t, h, c, r] with the t=1
  k-tile plane zeroed (Pool memsets), Vaug [128 r, h, jc, 65] (ones col).
  scores (fp8e4 DoubleRow, 0.5 cyc/col): lhsT = Kt chunk [64, 2, 128],
    rhs = Q span via a stride-0 t dim [64, (0,2), 512] -> psum
    [128 j', 512 i'] of raw q.k (2x the fp16 column rate; the zero
    k-tile pads contraction 64 -> 128).
  exp into fp8e4 E directly: ACT exact exp (scale=1/8, fp8 out) for 5
    of 8 groups per (h, i'-tile); DVE int8 Schraudolph for {2,4,6}
    (bits8 = round(s*8*log2e + 56), int8 bitcast = e4m3(e^s); the e4m3
    quantization dominates either path and cancels through the softmax
    normalization: end-to-end ~1.4e-3).
  PV (fp8 DoubleRow, contraction 256/matmul): ctx[i', 65] += E-pair^T @
    Vaug-pair over 8 k-tile pairs, 4x fewer PE cycles and half the
    instructions of the f16 version; col 64 = denominator.  DR matmul
    outputs MUST land on 512B-aligned psum offsets (sub-bank offsets
    wedge the exec unit - found empirically), so pc is [128, 4, 128].
  normalize: DVE reciprocal + per-partition tensor_scalar multiply.
  head-pair transpose: PE is_transpose [128 i', (2 heads x d)] -> psum
    f16 -> ctxT [128 (par, d), c, pair, r].
  out-projection per s'-tile: 8 accumulating f16 matmuls + residual
    (x + bo host-folded); LayerNorm stats on DVE, (y-mu)*rstd on the
    otherwise-idle Pool engine, affine on Pool/DVE, rstd on ACT.

Scheduling (tuned against the concourse TimelineSim cost model, the
grading metric in this axon container): with the PE at ~39% busy after
the fp8 move, the kernel is an ACT/DVE exp race (ACT 195.8us busy, DVE
189us, 256 exp groups of [128, 1024] split 5:3).  Three score buffers
(spool3: scores 3x2 psum banks + ctx/outproj 1 + aux 1 = 8) break the
exp->matmul->exp WAR chain of the 2-buffer rotation; PV/normalize/
transpose/out-projection defer ONE stage and pump between the next
head's score groups; projections for pair p+1 ride the same pump as
fillers; each pair's four LayerNorm output tiles leave in one batched
DMA.  222.4us end-to-end (fp16 predecessor: 260.7us, stub: 305us).
"""

import numpy as np

import concourse.bass as bass
import concourse.tile as tile
from concourse import mybir
from concourse.bass_utils import run_bass_kernel_spmd

F32 = mybir.dt.float32
F16 = mybir.dt.float16
F8 = mybir.dt.float8e4
I8 = mybir.dt.int8
A = mybir.AluOpType
AF = mybir.ActivationFunctionType
DR = mybir.MatmulPerfMode.DoubleRow

S = 2048
DIN = 64
H = 8
HD = 512
NT = 16          # s'-tiles of 128
P = 128
LN_EPS = 1e-5

# int8 Schraudolph fast-exp into fp8e4 bits: bits = round(s*8*log2e + 56)
# (s = raw_score / 8; slope folded with the 1/sqrt(D) scale below)
SCH_A = (8.0 / np.log(2.0)) * 0.125
SCH_B = 56.0
EXP_SCALE = 0.125         # ACT exact exp: exp(raw_score / 8)

# per-window DVE score-group index sets (cycled): which of the 8 exp
# groups per (head, i'-tile) run as DVE Schraudolph vs exact ACT exp
EXP_DVE_PAT = [{2, 4, 6}]
# ACT takes 1 of every EVAC_ACT_MOD psum evacuations (jitter absorber)
EVAC_ACT_MOD = 3
# these normalize sub-tiles run on ACT (Copy, scale=1/den) instead of DVE
NORM_ACT_SUBS = {3}
# pumps to run after emitting score-group g (index 0..7)
PUMP_CADENCE = [3, 2, 1, 1, 3, 1, 1, 2]

DEFAULT_CFG = dict(
    pat=EXP_DVE_PAT, evac_act_mod=EVAC_ACT_MOD, norm_act_subs=NORM_ACT_SUBS,
    pump=PUMP_CADENCE, spool3=True, warm_evac=10)

_CACHE = {}

# walrus in this container accepts at most 1 sync-wait command per engine
# instruction and at most 2 per EventSemaphore. Tile packs every dependency
# onto the consuming instruction, so hoist the excess onto standalone
# EventSemaphore instructions inserted just before it (same engine stream).
_MAX_EV_WAITS = 2


def _legalize_sync_waits(nc, keep=1):
    n_fixed = 0
    for function in nc.m.functions:
        for block in function.blocks:
            out = []
            changed = False
            for inst in block.instructions:
                si = getattr(inst, "sync_info", None)
                waits = list(si.on_wait) if (si is not None and si.on_wait) else []
                if len(waits) > keep and not isinstance(
                        inst, mybir.InstEventSemaphore):
                    extra = waits[:-keep] if keep else waits
                    kept = waits[-keep:] if keep else []
                    for ci in range(0, len(extra), _MAX_EV_WAITS):
                        ev = mybir.InstEventSemaphore(
                            name=f"{inst.name}-w{ci}", ins=[], outs=[],
                            sync_info=mybir.SyncInfo(
                                on_wait=extra[ci:ci + _MAX_EV_WAITS],
                                on_update=[]),
                        )
                        ev.engine = inst.engine
                        out.append(ev)
                    inst.sync_info = mybir.SyncInfo(
                        on_wait=kept, on_update=list(si.on_update))
                    changed = True
                    n_fixed += 1
                out.append(inst)
            if changed:
                block.instructions = out
    return n_fixed


def _build(cfg=None):
    cfg = {**DEFAULT_CFG, **(cfg or {})}
    _pat = cfg["pat"]
    _evac_act_mod = cfg["evac_act_mod"]
    _norm_act_subs = cfg["norm_act_subs"]
    _pump_cad = cfg["pump"]
    _spool3 = cfg.get("spool3", False)
    _yn_pool = cfg.get("yn_pool", True)
    _q_stride0 = cfg.get("q_stride0", True)
    _npairs = cfg.get("npairs", 4)
    _pool_memset = cfg.get("pool_memset", True)
    _defer2 = cfg.get("defer2", False)
    _warm_evac = cfg.get("warm_evac", 0)
    _warm_split = cfg.get("warm_split", False)
    _tevac_pair = cfg.get("tevac_pair", False)
    _pump_first = cfg.get("pump_first", None)
    _pump_last = cfg.get("pump_last", None)
    _fp8_scores = cfg.get("fp8_scores", True)
    _fp8_pv = cfg.get("fp8_pv", True)
    _fqk = F8 if _fp8_scores else F16
    _fe = F8 if _fp8_pv else F16
    nc = bass.Bass()

    xt_d = nc.dram_tensor("xt", [65, S], F16, kind="ExternalInput")
    wq_d = nc.dram_tensor("wq", [65, HD], F16, kind="ExternalInput")
    wk_d = nc.dram_tensor("wk", [65, HD], F16, kind="ExternalInput")
    wv_d = nc.dram_tensor("wv", [65, HD], F16, kind="ExternalInput")
    wo_d = nc.dram_tensor("wo", [P, H, DIN], F16, kind="ExternalInput")
    id_d = nc.dram_tensor("ident", [P, P], F16, kind="ExternalInput")
    x_d = nc.dram_tensor("x", [P, NT, DIN], F32, kind="ExternalInput")
    gam_d = nc.dram_tensor("gamma", [P, DIN], F32, kind="ExternalInput")
    bet_d = nc.dram_tensor("beta", [P, DIN], F32, kind="ExternalInput")
    y_d = nc.dram_tensor("y", [S, DIN], F32, kind="ExternalOutput")

    with tile.TileContext(nc) as tc:
        with (
            tc.tile_pool(name="consts", bufs=1) as consts,
            tc.tile_pool(name="spool", bufs=(3 if _spool3 else 2),
                         space=bass.MemorySpace.PSUM) as spool,
            tc.tile_pool(name="cpool", bufs=(1 if _spool3 else 2),
                         space=bass.MemorySpace.PSUM) as cpool,
            tc.tile_pool(name="aux", bufs=(1 if _spool3 else 2),
                         space=bass.MemorySpace.PSUM) as aux,
            tc.tile_pool(name="epool", bufs=cfg.get("ebufs", 5)) as epool,
            tc.tile_pool(name="cspool", bufs=cfg.get("csbufs", 3)) as cspool,
            tc.tile_pool(name="lpool", bufs=cfg.get("lbufs", 5)) as lpool,
        ):
            eps_t = consts.tile([P, 1], F32)
            nc.vector.memset(eps_t, LN_EPS)
            # trigger the ACT Exp table load during the prologue
            warm = consts.tile([P, 1], F32)
            nc.scalar.activation(warm[:], eps_t[:], AF.Exp)

            # DMA order gates the critical path: first K/Q projections.
            xt = consts.tile([65, S], F16)
            nc.sync.dma_start(xt[:, 0:HD], xt_d[:, 0:HD])
            wk_sb = consts.tile([65, HD], F16)
            nc.sync.dma_start(wk_sb[:, 0:P], wk_d[:, 0:P])
            wq_sb = consts.tile([65, HD], F16)
            nc.sync.dma_start(wq_sb[:, 0:P], wq_d[:, 0:P])
            nc.sync.dma_start(wk_sb[:, P:], wk_d[:, P:])
            nc.sync.dma_start(wq_sb[:, P:], wq_d[:, P:])
            wv_sb = consts.tile([65, HD], F16)
            nc.sync.dma_start(wv_sb[:], wv_d[:])
            nc.sync.dma_start(xt[:, HD:], xt_d[:, HD:])
            ident = consts.tile([P, P], F16)
            nc.sync.dma_start(ident[:], id_d[:])
            wo_sb = consts.tile([P, H, DIN], F16)
            nc.sync.dma_start(wo_sb[:], wo_d[:])
            x_res = consts.tile([P, NT, DIN], F32)
            nc.sync.dma_start(x_res[:], x_d[:])
            gamma_b = consts.tile([P, DIN], F32)
            nc.sync.dma_start(gamma_b[:], gam_d[:])
            beta_b = consts.tile([P, DIN], F32)
            nc.sync.dma_start(beta_b[:], bet_d[:])

            # Qt[d, h, c, r] fp8; Kt[d, t, h, c, r] fp8 with the t=1 plane
            # zeroed (DoubleRow k-tile padding; Q reads a stride-0 t dim
            # instead so it needs no pad plane). Vaug[r, h, jc, 65] fp8
            # with ones col 64.
            _ms_eng = nc.gpsimd if _pool_memset else nc.vector
            if not _fp8_scores:
                Qt = consts.tile([DIN, H, 8, 256], F16)
                Kt = consts.tile([DIN, 1, H, 8, 256], F16)
            elif _q_stride0:
                Qt = consts.tile([DIN, H, 8, 256], F8)
                Kt = consts.tile([DIN, 2, H, 8, 256], F8)
            else:
                Qt = consts.tile([DIN, 2, H, 8, 256], F8)
                for hh in range(H):
                    _ms_eng.memset(Qt[:, 1, hh], 0.0)
                Kt = consts.tile([DIN, 2, H, 8, 256], F8)
            if _fp8_scores:
                for hh in range(H):
                    _ms_eng.memset(Kt[:, 1, hh], 0.0)
            Vaug = consts.tile([P, H, NT, 65], _fe)
            _ms_eng.memset(Vaug[:, :, :, 64:65], 1.0)
            ctxT = consts.tile([P, 8, 4, 256], F16)

            y_all = consts.tile([P, NT, DIN], F32)
            mv_all = consts.tile([P, NT, 2], F32)
            rstd_all = consts.tile([P, NT], F32)
            lnv = consts.tile([P, NT], F32)

            # exp assignment: ACT exact vs DVE Schraudolph, fixed smooth
            # per-window patterns (tuned against the cost-model timeline)
            exp_n = [0]

            def exp_group(E_t, jc0, njc, ps):
                g = jc0 // 2
                w = exp_n[0]
                if g == 0:
                    exp_n[0] += 1
                nd = _pat[w % len(_pat)]
                dve = g in nd
                dst = E_t[:, jc0:jc0 + njc, :]
                if dve and _fp8_pv:
                    nc.vector.tensor_scalar(
                        dst.bitcast(I8), ps[:, 0:njc, :],
                        scalar1=SCH_A, scalar2=SCH_B, op0=A.mult, op1=A.add)
                elif dve:
                    nc.vector.tensor_scalar(
                        dst.bitcast(mybir.dt.int16), ps[:, 0:njc, :],
                        scalar1=SCH_A * 128.0, scalar2=15360.0 - 64.0,
                        op0=A.mult, op1=A.add)
                else:
                    nc.scalar.activation(
                        dst, ps[:, 0:njc, :], AF.Exp, scale=EXP_SCALE)

            evac_flip = [1]

            def _evac_copy(dst, src, p):
                # steady-state evacuations live on DVE so the ACT stream is
                # a homogeneous exp pipeline; pair-0 (prologue, ACT idle)
                # alternates to halve the startup critical path
                mod = 2 if evac_flip[0] <= _warm_evac else _evac_act_mod
                if mod and evac_flip[0] % mod == 0:
                    evac_flip[0] += 1
                    nc.scalar.activation(dst, src, AF.Copy)
                else:
                    evac_flip[0] += 1
                    nc.vector.tensor_copy(dst, src)

            def qk_proj_half(w_sb, dst_fn, p, m, hh):
                """warm-path: single head-half (256 xt cols) -> psum
                [(cc, d) 128, 256]; halves the prologue critical chain."""
                ps = aux.tile([P, HD], F32, tag="aux")
                nc.tensor.matmul(
                    ps[:, 0:256], w_sb[:, m * P:(m + 1) * P],
                    xt[:, (2 * p + hh) * 256:(2 * p + hh + 1) * 256],
                    start=True, stop=True)
                src = ps[:, 0:256].rearrange("(cc d) r -> cc d r", d=DIN)

                def evac(cc):
                    _evac_copy(dst_fn(p, m, cc)[:, hh, :], src[cc], p)
                return [lambda cc=cc: evac(cc) for cc in range(2)]

            def qk_proj(w_sb, dst_fn, p, m):
                """col-chunk m (c = 2m, 2m+1) x row-chunk p (heads 2p, 2p+1)
                -> psum [(cc, d) 128, (hh, r) 512]; returns evac closures."""
                ps = aux.tile([P, HD], F32, tag="aux")
                nc.tensor.matmul(
                    ps[:], w_sb[:, m * P:(m + 1) * P],
                    xt[:, p * HD:(p + 1) * HD], start=True, stop=True)
                src = ps[:].rearrange("(cc d) (hh r) -> cc d hh r", d=DIN, r=256)

                def evac(cc):
                    _evac_copy(dst_fn(p, m, cc), src[cc], p)
                return [lambda cc=cc: evac(cc) for cc in range(2)]

            def v_proj(p, k):
                """row-chunk p*512 + k*128 = head 2p + k//2, r-half k%2."""
                st = 4 * p + k
                h, half = st // 2, st % 2
                ps = aux.tile([P, HD], F32, tag="aux")
                nc.tensor.matmul(
                    ps[:], xt[:, st * P:(st + 1) * P], wv_sb[:],
                    start=True, stop=True)
                dst = bass.AP(
                    tensor=Vaug.tensor,
                    offset=Vaug.offset + (h * NT + half) * 65,
                    ap=[[Vaug.ap[0][0], P], [2 * 65, 8], [1, DIN]],
                )
                _evac_copy(
                    dst, ps[:].rearrange("p (c d) -> p c d", d=DIN), p)

            def proj_pair(p):
                """work-items producing Qt/Kt/Vaug for heads 2p, 2p+1.
                Matmul and each evacuation are separate items so the pump
                spaces the DVE copies out (no burst stalling exp service)."""
                work = []
                cells = {}

                def k_dst(p, m, cc):
                    return Kt[:, 0, 2 * p:2 * p + 2, 2 * m + cc, :]

                def q_dst(p, m, cc):
                    if not _fp8_scores or _q_stride0:
                        return Qt[:, 2 * p:2 * p + 2, 2 * m + cc, :]
                    return Qt[:, 0, 2 * p:2 * p + 2, 2 * m + cc, :]

                def add_qk(src, w_sb, dst_fn):
                    for m in range(4):
                        if _warm_split and p == 0 and m == 0:
                            for hh in range(2):
                                def mm_item(m=m, hh=hh, w_sb=w_sb,
                                            dst_fn=dst_fn, src=src):
                                    cells[(src, m, hh)] = qk_proj_half(
                                        w_sb, dst_fn, p, m, hh)
                                work.append(mm_item)
                                for cc in range(2):
                                    work.append(
                                        lambda m=m, hh=hh, cc=cc, src=src:
                                        cells[(src, m, hh)][cc]())
                            continue
                        def mm_item(m=m, w_sb=w_sb, dst_fn=dst_fn, src=src):
                            cells[(src, m)] = qk_proj(w_sb, dst_fn, p, m)
                        work.append(mm_item)
                        for cc in range(2):
                            work.append(
                                lambda m=m, cc=cc, src=src:
                                cells[(src, m)][cc]())
                # K first (score-group g needs K block c_j = g), then V
                # (PV pops with pvq priority one stage later), then Q
                # (blocks c >= 2 are only read from it=1 onward)
                add_qk("k", wk_sb, k_dst)
                for k in range(4):
                    work.append(lambda k=k: v_proj(p, k))
                add_qk("q", wq_sb, q_dst)
                return work

            def attention_scores(h, it, E_t, pump):
                """emit the score-groups + exp for (h, it); deferred work
                items (prev head's PV etc.) are pumped between groups."""
                cad = _pump_cad
                if h == 0 and it == 0 and _pump_first is not None:
                    cad = _pump_first
                elif h == H - 1 and it == 3 and _pump_last is not None:
                    cad = _pump_last
                if not _fp8_scores:
                    rhs_q = Qt[:, h, 2 * it:2 * it + 2, :]
                elif _q_stride0:
                    qs = Qt[:, h, 2 * it:2 * it + 2, :]
                    # stride-0 k-tile dim: DoubleRow sums K's t=0 (data)
                    # and t=1 (zeros) planes against the same Q span
                    rhs_q = bass.AP(
                        tensor=qs.tensor, offset=qs.offset,
                        ap=[list(qs.ap[0]), [0, 2]]
                        + [list(x) for x in qs.ap[1:]])
                else:
                    rhs_q = Qt[:, :, h, 2 * it:2 * it + 2, :]
                for g in range(8):
                    jc0 = 2 * g
                    ps = spool.tile([P, 2, HD], F32, tag="sc")
                    for jg in range(2):
                        jc = jc0 + jg
                        if _fp8_scores:
                            nc.tensor.matmul(
                                ps[:, jg, :],
                                Kt[:, :, h, jc // 2,
                                   (jc % 2) * P:(jc % 2) * P + P],
                                rhs_q, start=True, stop=True, perf_mode=DR)
                        else:
                            nc.tensor.matmul(
                                ps[:, jg, :],
                                Kt[:, 0, h, jc // 2,
                                   (jc % 2) * P:(jc % 2) * P + P],
                                rhs_q, start=True, stop=True)
                    exp_group(E_t, jc0, 2, ps)
                    for _ in range(cad[g]):
                        pump()

            def pv_items(h, it, E_t, cs):
                """deferred PV + normalize for (h, it), run one stage later
                (during the next head's score-groups) so the PE's PV block
                overlaps with fresh scores feeding the exp engines."""
                par = h % 2
                cell = {}

                def pv_sub(sub):
                    if sub == 0:
                        # sub stride padded to 128 f32 (512 B): DoubleRow
                        # matmul outputs fault on non-512B-aligned PSUM
                        # offsets (found empirically; 65-f32 stride wedges
                        # the exec unit)
                        pc = cpool.tile([P, 4, P], F32, tag="ctx")
                        cell["pc"] = pc
                    pc = cell["pc"]
                    if _fp8_pv:
                        for jc2 in range(NT // 2):
                            nc.tensor.matmul(
                                pc[:, sub, 0:65],
                                E_t[:, 2 * jc2:2 * jc2 + 2,
                                    sub * P:(sub + 1) * P],
                                Vaug[:, h, 2 * jc2:2 * jc2 + 2, :],
                                start=(jc2 == 0), stop=(jc2 == NT // 2 - 1),
                                perf_mode=DR)
                    else:
                        for jc in range(NT):
                            nc.tensor.matmul(
                                pc[:, sub, 0:65],
                                E_t[:, jc, sub * P:(sub + 1) * P],
                                Vaug[:, h, jc, :],
                                start=(jc == 0), stop=(jc == NT - 1))

                def norm():
                    pc = cell["pc"]
                    rd = lpool.tile([P, 4, 1], F32, tag="rd")
                    nc.vector.reciprocal(rd[:], pc[:, :, 64:65])
                    for sub in range(4):
                        dst = cs[:, sub, par * DIN:(par + 1) * DIN]
                        if sub in _norm_act_subs:
                            nc.scalar.activation(
                                dst, pc[:, sub, 0:DIN], AF.Copy,
                                scale=rd[:, sub, :])
                        else:
                            nc.vector.tensor_scalar_mul(
                                dst, pc[:, sub, 0:DIN], rd[:, sub, :])

                return ([lambda sub=sub: pv_sub(sub) for sub in range(4)],
                        norm)

            def transpose_pair(pair, it, cs):
                if not _tevac_pair:
                    for sub in range(4):
                        tp32 = aux.tile([P, HD], F32, tag="aux")
                        tp = tp32[:, 0:DIN].bitcast(F16)
                        nc.tensor.transpose(tp, cs[:, sub, :], ident[:])
                        c = 2 * it + sub // 2
                        half = sub % 2
                        dst = ctxT[:, c, pair, half * P:(half + 1) * P]
                        if sub == 3:
                            nc.scalar.activation(dst, tp, AF.Copy)
                        else:
                            nc.vector.tensor_copy(dst, tp)
                    return
                # paired: 2 transposes share one aux tile at 512B-aligned
                # f32 offsets (0 and 128); one 256-wide f16 evac drains both
                for hp in range(2):
                    tp32 = aux.tile([P, HD], F32, tag="aux")
                    for k in range(2):
                        tp = tp32[:, k * P:k * P + DIN].bitcast(F16)
                        nc.tensor.transpose(
                            tp, cs[:, 2 * hp + k, :], ident[:])
                    c = 2 * it + hp
                    dst = ctxT[:, c, pair, :]
                    srcap = tp32[:].bitcast(F16)
                    src = bass.AP(
                        tensor=srcap.tensor, offset=srcap.offset,
                        ap=[list(srcap.ap[0]), [2 * P, 2], [1, P]])
                    if hp == 1:
                        nc.scalar.activation(dst, src, AF.Copy)
                    else:
                        nc.vector.tensor_copy(dst, src)

            def outproj(st):
                h = st // 2
                pair, par = h // 2, h % 2
                b = par * DIN
                if _spool3:
                    po32 = cpool.tile([P, 4, P], F32, tag="ctx")
                    po = po32[:, 0, 0:DIN]
                else:
                    po32 = aux.tile([P, HD], F32, tag="aux")
                    po = po32[:, 0:DIN]
                for c in range(8):
                    nc.tensor.matmul(
                        po, ctxT[b:b + DIN, c, pair,
                                 (st % 2) * P:(st % 2) * P + P],
                        wo_sb[b:b + DIN, c, :],
                        start=(c == 0), stop=(c == 7))
                nc.vector.tensor_tensor(
                    y_all[:, st, :], po, x_res[:, st, :], A.add)
                stats = lpool.tile([P, 6], F32, tag="st")
                nc.vector.bn_stats(stats[:], y_all[:, st, :])
                nc.vector.bn_aggr(mv_all[:, st, :], stats[:])

            def finalize(st0, st1, tail=False):
                # on the drain tail DVE is idle: run the affine there instead
                # of GPSIMD to shorten the serial chain
                tt = nc.vector.tensor_tensor if tail else nc.gpsimd.tensor_tensor
                nc.scalar.activation(
                    lnv[:, st0:st1], mv_all[:, st0:st1, 1], AF.Ln,
                    bias=eps_t[:])
                nc.scalar.activation(
                    rstd_all[:, st0:st1], lnv[:, st0:st1], AF.Exp,
                    scale=-0.5)
                yo4 = lpool.tile([P, 4, DIN], F32, tag="yo4", bufs=2)
                for st in range(st0, st1):
                    yn = lpool.tile([P, DIN], F32, tag="yn")
                    # SBUF-only op: runs on the near-idle Pool engine
                    eng = nc.gpsimd if _yn_pool else nc.vector
                    eng.tensor_scalar(
                        yn[:], y_all[:, st, :],
                        scalar1=mv_all[:, st, 0:1],
                        scalar2=rstd_all[:, st:st + 1],
                        op0=A.subtract, op1=A.mult)
                    tt(yn[:], yn[:], gamma_b[:], A.mult)
                    tt(yo4[:, st - st0, :], yn[:], beta_b[:], A.add)
                # one batched DMA per pair: the 4 per-st stores serialized
                # ~700ns of HWDGE generation each on the drain tail
                nc.sync.dma_start(
                    y_d[st0 * P:(st0 + 4) * P, :].rearrange(
                        "(s p) d -> p s d", s=4),
                    yo4[:])

            from collections import deque
            pvq = deque()       # deferred PV/normalize/transpose, FIFO
            carry1 = []         # items delayed one stage
            carry2 = []         # (defer2 mode) two-stage carry
            fillers = deque()   # proj / outproj / finalize work

            def pump():
                if pvq:
                    pvq.popleft()()
                elif fillers:
                    fillers.popleft()()

            # only the K/Q column-chunks the first score-groups need run up
            # front; everything else (rest of pair-0 proj, later pairs'
            # projections) is sprinkled through the attention loop as
            # fillers, in dependency (FIFO) order.
            p0 = proj_pair(0)
            _qo = 19 if _warm_split else 16   # index of first Q item
            for w in p0[0:3] + p0[_qo:_qo + 3]:
                w()
            fillers.extend(p0[3:_qo])
            fillers.extend(p0[_qo + 3:])
            for pair in range(_npairs):
                if pair < 3:
                    fillers.extend(proj_pair(pair + 1))
                for it in range(4):
                    cs = cspool.tile([P, 4, P], F16, tag="cs")
                    for par in range(2):
                        h = 2 * pair + par
                        E_t = epool.tile([P, NT, HD], _fe, tag="E")
                        attention_scores(h, it, E_t, pump)
                        subs, norm = pv_items(h, it, E_t, cs)
                        # PV right away; with the fp8 DoubleRow PV the PE
                        # finishes each block quickly, so normalize /
                        # transposes / out-projections follow just ONE
                        # stage later (shorter drain tail, earlier filler
                        # supply for the exp engines)
                        if _defer2:
                            pvq.extend(carry2)
                            carry2.clear()
                            carry2.extend(carry1)
                            carry1.clear()
                        else:
                            pvq.extend(carry1)
                            carry1.clear()
                        pvq.extend(subs)
                        carry1.append(norm)
                        if par == 1:
                            carry1.append(
                                lambda pair=pair, it=it, cs=cs:
                                transpose_pair(pair, it, cs))
                            if it == 3:
                                for st in range(4 * pair, 4 * pair + 4):
                                    carry1.append(lambda st=st: outproj(st))
                                carry1.append(
                                    lambda pair=pair:
                                    finalize(4 * pair, 4 * pair + 4,
                                             tail=(pair == _npairs - 1)))
                while fillers:
                    fillers.popleft()()
            pvq.extend(carry2)
            pvq.extend(carry1)
            carry1.clear()
            carry2.clear()
            while pvq or fillers:
                pump()

    return nc


def _get_nc():
    if "nc" not in _CACHE:
        nc = _build()
        _legalize_sync_waits(nc)
        _CACHE["nc"] = nc
    return _CACHE["nc"]


def _prep_in_maps(x, Wq, bq, Wk, bk, Wv, bv, Wo, bo, gamma, beta):
    f32, f16 = np.float32, np.float16
    wq65 = np.concatenate(
        [np.asarray(Wq, f32), np.asarray(bq, f32)[None, :]], axis=0).astype(f16)
    wk65 = np.concatenate(
        [np.asarray(Wk, f32), np.zeros((1, HD), f32)], axis=0).astype(f16)
    wv65 = np.concatenate(
        [np.asarray(Wv, f32), np.asarray(bv, f32)[None, :]], axis=0).astype(f16)
    # wo[par*64 + d, c, dout] = Wo[c*64 + d, dout], both par halves
    wo3 = np.asarray(Wo, f32).astype(f16).reshape(H, DIN, DIN) \
        .transpose(1, 0, 2)
    wo2 = np.concatenate([wo3, wo3], axis=0).copy()
    ident = np.eye(P, dtype=f16)
    gb = np.ascontiguousarray(np.broadcast_to(np.asarray(gamma, f32), (P, DIN)))
    bb = np.ascontiguousarray(np.broadcast_to(np.asarray(beta, f32), (P, DIN)))
    bo_f = np.asarray(bo, f32)

    in_maps = []
    B = x.shape[0]
    for b in range(B):
        xb = np.asarray(x[b], f32)
        x3 = np.ascontiguousarray(
            xb.reshape(NT, P, DIN).transpose(1, 0, 2)) + bo_f
        xt65 = np.concatenate(
            [xb.T, np.ones((1, S), f32)], axis=0).astype(f16)
        in_maps.append(dict(
            xt=xt65, wq=wq65, wk=wk65, wv=wv65, wo=wo2, ident=ident,
            x=x3, gamma=gb, beta=bb,
        ))
    return in_maps


def run(trace=False, **inputs):
    nc = _get_nc()
    in_maps = _prep_in_maps(**inputs)
    res = run_bass_kernel_spmd(
        nc, in_maps, core_ids=list(range(len(in_maps))), trace=trace,
    )
    B = len(in_maps)
    y = np.stack([res.results[b]["y"] for b in range(B)])[:, None]
    return np.asarray(y, np.float32), res


def kernel(**inputs):
    y, _ = run(trace=False, **inputs)
    return y

